# revision 13
# baseline (speedup 1.0000x reference)
"""Siddon DRR kernel for 8 Trainium2 NeuronCores.

Ray-parallel: 40000 rays split 8 ways. Volume (flipped, bf16) on every core
as 256 x-slab ap_gather tables (2 generations of 128 slabs, one per SBUF
partition). Per (ray, x-slab) alpha window there are at most 3 y/z plane
crossings for this geometry, so each window is exactly 4 Siddon intervals:
boundaries [t0,x1,x2,x3,t1] from a 2+2 merge of per-axis crossing candidates
(closed-form counts of arithmetic alpha sequences). Widths/midpoints/voxel
indices computed densely on DVE/ACT, voxel fetch via GPSIMD ap_gather (d=2
bf16 pairs), weighted 4-slot segmented reduce on DVE, cross-partition sum
via PE ones-matmul, per-ray ||sdd|| scale, DMA out.
"""
import sys
sys.path.insert(0, "/opt/trn_rl_repo")
import contextlib
import numpy as np

import concourse.bass as bass
import concourse.bacc as bacc
import concourse.tile as tile
import concourse.mybir as mybir
from concourse.bass_utils import run_bass_kernel_spmd
from ml_dtypes import bfloat16

F32 = np.float32
AOP = mybir.AluOpType
ACTF = mybir.ActivationFunctionType

H = W = 200
NXV = NYV = NZV = 256
DELX = DELY = 1.5
EPS = 1e-8
NCORES = 8
NRAY = H * W
NPC = NRAY // NCORES          # rays per core (5000)
RC = 96                       # rays per chunk
NCH = 54                      # chunks per generation
NPAD = RC * NCH               # padded rays per core (5184)
NP = 15                       # packed params per ray
MAGIC = F32(12582912.0)       # 2^23 + 2^22: round-to-nearest for |x|<2^22
BIG = F32(3.0e38)
NE = 32768                    # bf16 pairs per slab table

_cache = {}


def _host_geometry(spacing, sdr, rotations, translations):
    """Per-ray launch parameters, float32 (matches reference numerics)."""
    sp = np.asarray(spacing, F32)
    sdrf = F32(np.asarray(sdr).reshape(-1)[0])
    rot = np.asarray(rotations, F32)
    tr = np.asarray(translations, F32)
    th, ph, ga = rot[0]
    ct, st = np.cos(th, dtype=F32), np.sin(th, dtype=F32)
    cp, spn = np.cos(ph, dtype=F32), np.sin(ph, dtype=F32)
    cg, sg = np.cos(ga, dtype=F32), np.sin(ga, dtype=F32)
    Rz = np.array([[ct, -st, 0], [st, ct, 0], [0, 0, 1]], F32)
    Ry = np.array([[cp, 0, spn], [0, 1, 0], [-spn, 0, cp]], F32)
    Rx = np.array([[1, 0, 0], [0, cg, -sg], [0, sg, cg]], F32)
    Rm = (Rz @ Ry @ Rx).astype(F32)
    source3 = (sdrf * Rm[:, 0]).astype(F32)
    center3 = (-source3).astype(F32)
    basis = np.stack([Rm[:, 1], Rm[:, 2]]).astype(F32)
    source3 = source3 + tr[0]
    center3 = center3 + tr[0]
    t = (np.arange(-(H // 2), H // 2, dtype=F32) + F32(1.0)) * F32(DELX)
    s = (np.arange(-(W // 2), W // 2, dtype=F32) + F32(1.0)) * F32(DELY)
    coefs = np.stack(np.meshgrid(t, s, indexing="ij"), -1).reshape(-1, 2).astype(F32)
    target = (coefs @ basis + center3).astype(F32)
    sdd = (target - source3 + F32(EPS)).astype(F32)

    a0 = ((F32(0.0) - source3) / sdd).astype(F32)
    extent = (np.array([NXV, NYV, NZV], F32) * sp).astype(F32)
    a1 = ((extent - source3) / sdd).astype(F32)
    amin = np.minimum(a0, a1).max(-1).astype(F32)
    amax = np.maximum(a0, a1).min(-1).astype(F32)
    rnorm = np.sqrt((sdd * sdd).sum(-1)).astype(F32)

    hx, hy, hz = sp
    sx, sy, sz = source3
    p = np.zeros((NP, NRAY), F32)
    p[0] = sdd[:, 0]; p[1] = sdd[:, 1]; p[2] = sdd[:, 2]
    p[3] = amin; p[4] = amax
    Dy = np.where(sdd[:, 1] > 0, F32(1.0), F32(-1.0)).astype(F32)
    Dz = np.where(sdd[:, 2] > 0, F32(1.0), F32(-1.0)).astype(F32)
    p[5] = sdd[:, 1] * Dy / hy                       # alpha_y
    p[6] = sy * Dy / hy                              # beta_y
    p[7] = np.where(Dy > 0, F32(1.0), F32(257.0))    # offy
    p[8] = Dy
    p[9] = np.where(Dy > 0, F32(0.0), F32(256.0))    # m0y
    p[10] = sdd[:, 2] * Dz / hz
    p[11] = sz * Dz / hz
    p[12] = np.where(Dz > 0, F32(1.0), F32(257.0))
    p[13] = Dz
    p[14] = np.where(Dz > 0, F32(0.0), F32(256.0))
    scal = dict(hx=F32(hx), hy=F32(hy), hz=F32(hz),
                sx=F32(sx), sy=F32(sy), sz=F32(sz))
    return p, rnorm, scal


def _build(scal):
    hx, hy, hz = scal["hx"], scal["hy"], scal["hz"]
    sx, sy, sz = scal["sx"], scal["sy"], scal["sz"]
    inv_hy, inv_hz = 1.0 / hy, 1.0 / hz

    nc = bacc.Bacc("TRN2", target_bir_lowering=False, debug=False,
                   num_devices=1)
    vol_in = nc.dram_tensor("vol", [2, 128, NYV * NZV],
                            mybir.dt.bfloat16, kind="ExternalInput")
    par_in = nc.dram_tensor("par", [NCH, 128, NP * RC], mybir.dt.float32,
                            kind="ExternalInput")
    rno_in = nc.dram_tensor("rno", [1, NPAD], mybir.dt.float32,
                            kind="ExternalInput")
    msk_in = nc.dram_tensor("msk", [128, 16], mybir.dt.float32,
                            kind="ExternalInput")
    drr_out = nc.dram_tensor("drr", [1, NPAD], mybir.dt.float32,
                             kind="ExternalOutput")
    s_hbm = nc.dram_tensor("s_part", [2, 128, NPAD], mybir.dt.float32)

    with tile.TileContext(nc) as tc:
        with tc.tile_pool(name="pool", bufs=1) as pool, \
             tc.tile_pool(name="ppsum", bufs=2, space="PSUM") as ppsum:
            table = pool.tile([128, NYV * NZV], mybir.dt.bfloat16,
                              tag="table")
            jci = pool.tile([128, 1], mybir.dt.int32, tag="jci")
            jc0 = pool.tile([128, 1], mybir.dt.float32, tag="jc0")
            nc.gpsimd.iota(jci[:], pattern=[[0, 1]], base=0,
                           channel_multiplier=1)
            nc.vector.tensor_copy(jc0[:], jci[:])   # partition index p
            msk = pool.tile([128, 16], mybir.dt.float32, tag="msk")
            nc.sync.dma_start(msk[:], msk_in[:])

            def T(tag, w=1, dt=mybir.dt.float32):
                return pool.tile([128, w * RC], dt, tag=tag, name=tag)

            for g in range(2):
                nc.gpsimd.dma_start(table[:], vol_in[g])
                # per-partition window numerators: j = p + 128g
                njc0 = pool.tile([128, 1], mybir.dt.float32, tag="njc0")
                njc1 = pool.tile([128, 1], mybir.dt.float32, tag="njc1")
                # njc0 = j*hx - sx ; njc1 = (j+1)*hx - sx
                nc.vector.tensor_scalar(njc0[:], jc0[:], float(hx),
                                        float(128.0 * g * hx - sx),
                                        AOP.mult, AOP.add)
                nc.vector.tensor_scalar(njc1[:], jc0[:], float(hx),
                                        float(128.0 * g * hx - sx + hx),
                                        AOP.mult, AOP.add)
                for ch in range(NCH):
                    pt = pool.tile([128, NP * RC], mybir.dt.float32, tag="pt")
                    nc.sync.dma_start(pt[:], par_in[ch])
                    P = lambda i: pt[:, i * RC:(i + 1) * RC]
                    sddy, sddz = P(1), P(2)
                    amin, amax = P(3), P(4)
                    ay_, by_, offy, Dy, m0y = P(5), P(6), P(7), P(8), P(9)
                    az_, bz_, offz, Dz, m0z = P(10), P(11), P(12), P(13), P(14)

                    rsx, rsy, rsz = T("rsx"), T("rsy"), T("rsz")
                    nc.vector.reciprocal(rsx[:], P(0))
                    nc.vector.reciprocal(rsy[:], sddy)
                    nc.vector.reciprocal(rsz[:], sddz)

                    bb = pool.tile([128, 5 * RC], mybir.dt.float32, tag="bb")
                    t0 = bb[:, 0 * RC:1 * RC]
                    t1 = bb[:, 4 * RC:5 * RC]
                    axj, axj1 = T("axj"), T("axj1")
                    # axj = (j*hx-sx) * (1/sddx): per-partition scalar * tensor
                    nc.vector.tensor_scalar(axj[:], rsx[:], njc0[:, 0:1], None,
                                            AOP.mult)
                    nc.vector.tensor_scalar(axj1[:], rsx[:], njc1[:, 0:1],
                                            None, AOP.mult)
                    nc.vector.scalar_tensor_tensor(t0, axj[:], 1.0,
                                                   axj1[:], AOP.bypass,
                                                   AOP.min)
                    nc.vector.scalar_tensor_tensor(t0, t0, 1.0, amin,
                                                   AOP.bypass, AOP.max)
                    nc.vector.scalar_tensor_tensor(t1, axj[:], 1.0,
                                                   axj1[:], AOP.bypass,
                                                   AOP.max)
                    nc.vector.scalar_tensor_tensor(t1, t1, 1.0, amax,
                                                   AOP.bypass, AOP.min)

                    def counts(tt, al, be, off, outtag):
                        """C(t) = clamp(floor(t*al+be) + off, 0, 257)"""
                        gv, rv, fxs = T("cgv"), T("crv"), T("cfx")
                        nc.vector.scalar_tensor_tensor(gv[:], tt, 1.0,
                                                       al, AOP.bypass,
                                                       AOP.mult)
                        nc.vector.scalar_tensor_tensor(gv[:], gv[:], 1.0,
                                                       be, AOP.bypass, AOP.add)
                        nc.vector.tensor_scalar(rv[:], gv[:], float(MAGIC), float(MAGIC),
                                                AOP.add, AOP.subtract)
                        nc.vector.scalar_tensor_tensor(fxs[:], rv[:], 1.0,
                                                       gv[:], AOP.bypass,
                                                       AOP.is_gt)
                        out = T(outtag)
                        nc.vector.scalar_tensor_tensor(out[:], rv[:], 1.0,
                                                       fxs[:], AOP.bypass,
                                                       AOP.subtract)
                        nc.vector.scalar_tensor_tensor(out[:], out[:],
                                                       1.0, off,
                                                       AOP.bypass, AOP.add)
                        nc.vector.tensor_scalar(out[:], out[:], 0.0,
                                                257.0, AOP.max, AOP.min)
                        return out

                    def axis_events(al, be, off, Dv, m0, rs, hh, ss, tg):
                        C0 = counts(t0, al, be, off, "C0")
                        C1 = counts(t1, al, be, off, "C1")
                        cnt, m1, m2 = T("cnt"), T("m1"), T("m2")
                        nc.vector.scalar_tensor_tensor(cnt[:], C1[:], 1.0,
                                                       C0[:], AOP.bypass,
                                                       AOP.subtract)
                        nc.vector.scalar_tensor_tensor(m1[:], C0[:], 1.0,
                                                       Dv, AOP.bypass,
                                                       AOP.mult)
                        nc.vector.scalar_tensor_tensor(m1[:], m1[:], 1.0,
                                                       m0, AOP.bypass, AOP.add)
                        nc.vector.scalar_tensor_tensor(m2[:], m1[:], 1.0,
                                                       Dv, AOP.bypass, AOP.add)
                        e1, e2 = T(tg + "e1"), T(tg + "e2")
                        gg = T("gg", 1, mybir.dt.uint8)
                        for (mm, ee, thr) in ((m1, e1, 0.5),
                                              (m2, e2, 1.5)):
                            nc.vector.tensor_scalar(ee[:], mm[:], float(hh), float(-ss),
                                                    AOP.mult, AOP.add)
                            nc.vector.scalar_tensor_tensor(ee[:], ee[:],
                                                           1.0, rs[:],
                                                           AOP.bypass,
                                                           AOP.mult)
                            # invalid event -> amax sentinel (loses every min,
                            # clips to t1 => zero-width slot). Exact overwrite.
                            nc.vector.tensor_scalar(gg[:], cnt[:], thr, None,
                                                    AOP.is_le)
                            nc.vector.copy_predicated(ee[:], gg[:], amax)
                        return e1, e2

                    e1, e2 = axis_events(ay_, by_, offy, Dy, m0y, rsy, hy, sy,
                                         "y")
                    f1, f2 = axis_events(az_, bz_, offz, Dz, m0z, rsz, hz, sz,
                                         "z")

                    x1 = bb[:, 1 * RC:2 * RC]
                    x2 = bb[:, 2 * RC:3 * RC]
                    x3 = bb[:, 3 * RC:4 * RC]
                    hilo, lohi = T("hilo"), T("lohi")
                    nc.vector.scalar_tensor_tensor(x1, e1[:], 1.0, f1[:],
                                                   AOP.bypass, AOP.min)
                    nc.vector.scalar_tensor_tensor(hilo[:], e1[:], 1.0,
                                                   f1[:], AOP.bypass, AOP.max)
                    nc.vector.scalar_tensor_tensor(lohi[:], e2[:], 1.0,
                                                   f2[:], AOP.bypass, AOP.min)
                    nc.vector.scalar_tensor_tensor(x2, hilo[:], 1.0,
                                                   lohi[:], AOP.bypass,
                                                   AOP.min)
                    nc.vector.scalar_tensor_tensor(x3, hilo[:], 1.0,
                                                   lohi[:], AOP.bypass,
                                                   AOP.max)
                    for xx in (x1, x2, x3):
                        nc.vector.scalar_tensor_tensor(xx, xx, 1.0, t0,
                                                       AOP.bypass, AOP.max)
                        nc.vector.scalar_tensor_tensor(xx, xx, 1.0, t1,
                                                       AOP.bypass, AOP.min)

                    wgt = T("wgt", 4)
                    mid = T("mid", 4)
                    bL = bb[:, 0:4 * RC]
                    bR = bb[:, RC:5 * RC]
                    nc.vector.scalar_tensor_tensor(wgt[:], bR, 1.0, bL,
                                                   AOP.bypass, AOP.subtract)
                    nc.vector.tensor_scalar(wgt[:], wgt[:], 0.0, None,
                                            AOP.max)
                    nc.vector.scalar_tensor_tensor(mid[:], bL, 1.0, bR,
                                                   AOP.bypass, AOP.add)
                    nc.vector.tensor_scalar(mid[:], mid[:], 0.5, None,
                                            AOP.mult)

                    mid3 = mid[:].rearrange("p (four r) -> p four r", four=4)

                    def slot_floor_idx(sdd_p, inv_h, ss, outtag):
                        """floor((ss + mid*sdd)/h) clip [0,255] on [128,4RC]"""
                        pos, fxs = T("spos", 4), T("sfx", 4)
                        pos3 = pos[:].rearrange("p (four r) -> p four r",
                                                four=4)
                        sdd_b = sdd_p.rearrange(
                            "p (one r) -> p one r", one=1).broadcast_to(
                            (128, 4, RC))
                        nc.vector.scalar_tensor_tensor(pos3, mid3, 1.0,
                                                       sdd_b, AOP.bypass,
                                                       AOP.mult)
                        nc.vector.tensor_scalar(pos[:], pos[:], float(ss), float(inv_h),
                                                AOP.add, AOP.mult)
                        rv = T(outtag, 4)
                        nc.vector.tensor_scalar(rv[:], pos[:], float(MAGIC), float(MAGIC),
                                                AOP.add, AOP.subtract)
                        nc.vector.scalar_tensor_tensor(fxs[:], rv[:], 1.0,
                                                       pos[:], AOP.bypass,
                                                       AOP.is_gt)
                        nc.vector.scalar_tensor_tensor(rv[:], rv[:], 1.0,
                                                       fxs[:], AOP.bypass,
                                                       AOP.subtract)
                        nc.vector.tensor_scalar(rv[:], rv[:], 0.0,
                                                255.0, AOP.max, AOP.min)
                        return rv

                    iy = slot_floor_idx(sddy, inv_hy, sy, "iy")
                    iz = slot_floor_idx(sddz, inv_hz, sz, "iz")

                    izh, odd = T("izh", 4), T("odd", 4)
                    zh = T("zh", 4)
                    nc.vector.tensor_scalar(zh[:], iz[:], 0.5, None, AOP.mult)
                    nc.vector.tensor_scalar(izh[:], zh[:], float(MAGIC),
                                            float(MAGIC), AOP.add,
                                            AOP.subtract)
                    zfx = T("zfx", 4)
                    nc.vector.scalar_tensor_tensor(zfx[:], izh[:], 1.0, zh[:],
                                                   AOP.bypass, AOP.is_gt)
                    nc.vector.scalar_tensor_tensor(izh[:], izh[:], 1.0,
                                                   zfx[:], AOP.bypass,
                                                   AOP.subtract)
                    nc.vector.scalar_tensor_tensor(odd[:], izh[:], -2.0,
                                                   iz[:], AOP.mult, AOP.add)
                    pairf = T("pairf", 4)
                    nc.vector.scalar_tensor_tensor(pairf[:], iy[:], 128.0,
                                                   izh[:], AOP.mult, AOP.add)
                    idx16 = pool.tile([128, 4 * RC], mybir.dt.int16,
                                      tag="idx16")
                    nc.vector.tensor_copy(idx16[:], pairf[:])

                    gt = pool.tile([128, 4 * RC, 16, 2], mybir.dt.bfloat16,
                                   tag="gt")
                    nc.gpsimd.ap_gather(
                        out_ap=gt[:].rearrange("p a b c -> p (a b c)"),
                        in_ap=table[:], idxs_ap=idx16[:],
                        channels=128, num_elems=NE, d=2, num_idxs=16 * 4 * RC)

                    acc = T("acc", 4)
                    nc.vector.memset(acc[:], 0.0)
                    dvq, vq, wlom, mq = (T("dvq", 4), T("vq", 4),
                                         T("wlom", 4), T("mq", 4))
                    for q in range(16):
                        lo_q = gt[:, :, q, 0]
                        hi_q = gt[:, :, q, 1]
                        nc.vector.scalar_tensor_tensor(dvq[:], hi_q, 1.0,
                                                       lo_q, AOP.bypass,
                                                       AOP.subtract)
                        nc.vector.scalar_tensor_tensor(vq[:], dvq[:], 1.0,
                                                       odd[:], AOP.bypass,
                                                       AOP.mult)
                        nc.vector.scalar_tensor_tensor(vq[:], vq[:], 1.0,
                                                       lo_q, AOP.bypass,
                                                       AOP.add)
                        nc.vector.tensor_scalar(wlom[:], wgt[:],
                                                msk[:, q:q + 1], None,
                                                AOP.mult)
                        nc.vector.scalar_tensor_tensor(mq[:], vq[:], 1.0,
                                                       wlom[:], AOP.bypass,
                                                       AOP.mult)
                        nc.vector.scalar_tensor_tensor(acc[:], acc[:], 1.0,
                                                       mq[:], AOP.bypass,
                                                       AOP.add)
                    sch = T("sch")
                    nc.vector.tensor_reduce(
                        sch[:],
                        acc[:].rearrange("p (four r) -> p r four", four=4),
                        axis=mybir.AxisListType.X, op=AOP.add)
                    nc.sync.dma_start(s_hbm[g, :, ch * RC:(ch + 1) * RC],
                                      sch[:])

            # DRR[r] = rnorm[r] * sum_p (s0[p,r] + s1[p,r])
            ones = pool.tile([128, 1], mybir.dt.float32, tag="ones")
            nc.vector.memset(ones[:], 1.0)
            CH2 = 432
            for nb in range(NPAD // CH2):
                sl = slice(nb * CH2, (nb + 1) * CH2)
                s0 = pool.tile([128, CH2], mybir.dt.float32, tag="s0")
                s1 = pool.tile([128, CH2], mybir.dt.float32, tag="s1")
                nc.sync.dma_start(s0[:], s_hbm[0, :, sl])
                nc.sync.dma_start(s1[:], s_hbm[1, :, sl])
                nc.vector.scalar_tensor_tensor(s0[:], s0[:], 1.0, s1[:],
                                               AOP.bypass, AOP.add)
                ps = ppsum.tile([1, CH2], mybir.dt.float32)
                nc.tensor.matmul(ps[:], ones[:], s0[:], start=True, stop=True)
                rnc = pool.tile([1, CH2], mybir.dt.float32, tag="rnc")
                nc.sync.dma_start(rnc[:], rno_in[:, sl])
                orow = pool.tile([1, CH2], mybir.dt.float32, tag="orow")
                nc.vector.scalar_tensor_tensor(orow[:], ps[:], 1.0,
                                               rnc[:], AOP.bypass, AOP.mult)
                nc.sync.dma_start(drr_out[:, sl], orow[:])

    nc.finalize()
    return nc


def kernel(volume, spacing, sdr, rotations, translations):
    vol = np.asarray(volume, F32)
    par, rnorm, scal = _host_geometry(spacing, sdr, rotations, translations)
    if "nc" not in _cache:
        _cache["nc"] = _build(scal)
    nc = _cache["nc"]

    volf = vol[::-1].astype(bfloat16)                    # reference flip
    vol_dev = np.ascontiguousarray(volf.reshape(2, 128, NYV * NZV))

    in_maps = []
    for c in range(NCORES):
        lo = c * NPC
        pc = np.zeros((NP, NPAD), F32)
        pc[:, :NPC] = par[:, lo:lo + NPC]
        pc[:, NPC:] = pc[:, :1]                          # pad with ray 0
        pch = np.ascontiguousarray(
            pc.reshape(NP, NCH, RC).transpose(1, 0, 2)).reshape(NCH, 1,
                                                                NP * RC)
        pch = np.ascontiguousarray(np.broadcast_to(pch, (NCH, 128, NP * RC)))
        rn = np.zeros((1, NPAD), F32)
        rn[0, :NPC] = rnorm[lo:lo + NPC]
        mk = (np.arange(128)[:, None] % 16 ==
              np.arange(16)[None, :]).astype(F32)
        in_maps.append({"vol": vol_dev, "par": pch, "rno": rn, "msk": mk})

    res = run_bass_kernel_spmd(nc, in_maps, list(range(NCORES)))
    drr = np.concatenate([res.results[c]["drr"][0, :NPC]
                          for c in range(NCORES)])
    return drr.reshape(1, H, W).astype(F32)


# revision 14
# speedup vs baseline: 5.9121x; 5.9121x over previous
"""Siddon DRR kernel for 8 Trainium2 NeuronCores.

Ray-parallel: 40000 rays split 8 ways. Volume (flipped, bf16) on every core
as 256 x-slab ap_gather tables (2 generations of 128 slabs, one per SBUF
partition). Per (ray, x-slab) alpha window there are at most 3 y/z plane
crossings for this geometry, so each window is exactly 4 Siddon intervals:
boundaries [t0,x1,x2,x3,t1] from a 2+2 merge of per-axis crossing candidates
(closed-form counts of arithmetic alpha sequences). Widths/midpoints/voxel
indices computed densely on DVE/ACT, voxel fetch via GPSIMD ap_gather (d=2
bf16 pairs), weighted 4-slot segmented reduce on DVE, cross-partition sum
via PE ones-matmul, per-ray ||sdd|| scale, DMA out.
"""
import sys
sys.path.insert(0, "/opt/trn_rl_repo")
import contextlib
import numpy as np

import concourse.bass as bass
import concourse.bacc as bacc
import concourse.tile as tile
import concourse.mybir as mybir
from concourse.bass_utils import run_bass_kernel_spmd
from ml_dtypes import bfloat16

F32 = np.float32
AOP = mybir.AluOpType
ACTF = mybir.ActivationFunctionType

H = W = 200
NXV = NYV = NZV = 256
DELX = DELY = 1.5
EPS = 1e-8
NCORES = 8
NRAY = H * W
NPC = NRAY // NCORES          # rays per core (5000)
RC = 96                       # rays per chunk
NCH = 54                      # chunks per generation
NPAD = RC * NCH               # padded rays per core (5184)
NP = 15                       # packed params per ray
MAGIC = F32(12582912.0)       # 2^23 + 2^22: round-to-nearest for |x|<2^22
BIG = F32(3.0e38)
NE = 32768                    # bf16 pairs per slab table

_cache = {}


def _host_geometry(spacing, sdr, rotations, translations):
    """Per-ray launch parameters, float32 (matches reference numerics)."""
    sp = np.asarray(spacing, F32)
    sdrf = F32(np.asarray(sdr).reshape(-1)[0])
    rot = np.asarray(rotations, F32)
    tr = np.asarray(translations, F32)
    th, ph, ga = rot[0]
    ct, st = np.cos(th, dtype=F32), np.sin(th, dtype=F32)
    cp, spn = np.cos(ph, dtype=F32), np.sin(ph, dtype=F32)
    cg, sg = np.cos(ga, dtype=F32), np.sin(ga, dtype=F32)
    Rz = np.array([[ct, -st, 0], [st, ct, 0], [0, 0, 1]], F32)
    Ry = np.array([[cp, 0, spn], [0, 1, 0], [-spn, 0, cp]], F32)
    Rx = np.array([[1, 0, 0], [0, cg, -sg], [0, sg, cg]], F32)
    Rm = (Rz @ Ry @ Rx).astype(F32)
    source3 = (sdrf * Rm[:, 0]).astype(F32)
    center3 = (-source3).astype(F32)
    basis = np.stack([Rm[:, 1], Rm[:, 2]]).astype(F32)
    source3 = source3 + tr[0]
    center3 = center3 + tr[0]
    t = (np.arange(-(H // 2), H // 2, dtype=F32) + F32(1.0)) * F32(DELX)
    s = (np.arange(-(W // 2), W // 2, dtype=F32) + F32(1.0)) * F32(DELY)
    coefs = np.stack(np.meshgrid(t, s, indexing="ij"), -1).reshape(-1, 2).astype(F32)
    target = (coefs @ basis + center3).astype(F32)
    sdd = (target - source3 + F32(EPS)).astype(F32)

    a0 = ((F32(0.0) - source3) / sdd).astype(F32)
    extent = (np.array([NXV, NYV, NZV], F32) * sp).astype(F32)
    a1 = ((extent - source3) / sdd).astype(F32)
    amin = np.minimum(a0, a1).max(-1).astype(F32)
    amax = np.maximum(a0, a1).min(-1).astype(F32)
    rnorm = np.sqrt((sdd * sdd).sum(-1)).astype(F32)

    hx, hy, hz = sp
    sx, sy, sz = source3
    p = np.zeros((NP, NRAY), F32)
    p[0] = sdd[:, 0]; p[1] = sdd[:, 1]; p[2] = sdd[:, 2]
    p[3] = amin; p[4] = amax
    Dy = np.where(sdd[:, 1] > 0, F32(1.0), F32(-1.0)).astype(F32)
    Dz = np.where(sdd[:, 2] > 0, F32(1.0), F32(-1.0)).astype(F32)
    p[5] = sdd[:, 1] * Dy / hy                       # alpha_y
    p[6] = sy * Dy / hy                              # beta_y
    p[7] = np.where(Dy > 0, F32(1.0), F32(257.0))    # offy
    p[8] = Dy
    p[9] = np.where(Dy > 0, F32(0.0), F32(256.0))    # m0y
    p[10] = sdd[:, 2] * Dz / hz
    p[11] = sz * Dz / hz
    p[12] = np.where(Dz > 0, F32(1.0), F32(257.0))
    p[13] = Dz
    p[14] = np.where(Dz > 0, F32(0.0), F32(256.0))
    scal = dict(hx=F32(hx), hy=F32(hy), hz=F32(hz),
                sx=F32(sx), sy=F32(sy), sz=F32(sz))
    return p, rnorm, scal


def _build(scal):
    hx, hy, hz = scal["hx"], scal["hy"], scal["hz"]
    sx, sy, sz = scal["sx"], scal["sy"], scal["sz"]
    inv_hy, inv_hz = 1.0 / hy, 1.0 / hz

    nc = bacc.Bacc("TRN2", target_bir_lowering=False, debug=False,
                   num_devices=1)
    vol_in = nc.dram_tensor("vol", [2, 128, NYV * NZV],
                            mybir.dt.bfloat16, kind="ExternalInput")
    par_in = nc.dram_tensor("par", [NCH, 128, NP * RC], mybir.dt.float32,
                            kind="ExternalInput")
    rno_in = nc.dram_tensor("rno", [1, NPAD], mybir.dt.float32,
                            kind="ExternalInput")
    msk_in = nc.dram_tensor("msk", [128, 16], mybir.dt.float32,
                            kind="ExternalInput")
    drr_out = nc.dram_tensor("drr", [1, NPAD], mybir.dt.float32,
                             kind="ExternalOutput")
    s_hbm = nc.dram_tensor("s_part", [2, 128, NPAD], mybir.dt.float32)

    with tile.TileContext(nc) as tc:
        with tc.tile_pool(name="pool", bufs=1) as pool, \
             tc.tile_pool(name="ppsum", bufs=2, space="PSUM") as ppsum:
            table = pool.tile([128, NYV * NZV], mybir.dt.bfloat16,
                              tag="table")
            jci = pool.tile([128, 1], mybir.dt.int32, tag="jci")
            jc0 = pool.tile([128, 1], mybir.dt.float32, tag="jc0")
            nc.gpsimd.iota(jci[:], pattern=[[0, 1]], base=0,
                           channel_multiplier=1)
            nc.vector.tensor_copy(jc0[:], jci[:])   # partition index p
            msk = pool.tile([128, 16], mybir.dt.float32, tag="msk")
            nc.sync.dma_start(msk[:], msk_in[:])

            def T(tag, w=1, dt=mybir.dt.float32):
                return pool.tile([128, w * RC], dt, tag=tag, name=tag)

            for g in range(2):
                nc.gpsimd.dma_start(table[:], vol_in[g])
                # per-partition window numerators: j = p + 128g
                njc0 = pool.tile([128, 1], mybir.dt.float32, tag="njc0")
                njc1 = pool.tile([128, 1], mybir.dt.float32, tag="njc1")
                # njc0 = j*hx - sx ; njc1 = (j+1)*hx - sx
                nc.vector.tensor_scalar(njc0[:], jc0[:], float(hx),
                                        float(128.0 * g * hx - sx),
                                        AOP.mult, AOP.add)
                nc.vector.tensor_scalar(njc1[:], jc0[:], float(hx),
                                        float(128.0 * g * hx - sx + hx),
                                        AOP.mult, AOP.add)
                for ch in range(NCH):
                    pt = pool.tile([128, NP * RC], mybir.dt.float32, tag="pt")
                    nc.sync.dma_start(pt[:], par_in[ch])
                    P = lambda i: pt[:, i * RC:(i + 1) * RC]
                    sddy, sddz = P(1), P(2)
                    amin, amax = P(3), P(4)
                    ay_, by_, offy, Dy, m0y = P(5), P(6), P(7), P(8), P(9)
                    az_, bz_, offz, Dz, m0z = P(10), P(11), P(12), P(13), P(14)

                    rsx, rsy, rsz = T("rsx"), T("rsy"), T("rsz")
                    nc.vector.reciprocal(rsx[:], P(0))
                    nc.vector.reciprocal(rsy[:], sddy)
                    nc.vector.reciprocal(rsz[:], sddz)

                    bb = pool.tile([128, 5 * RC], mybir.dt.float32, tag="bb")
                    t0 = bb[:, 0 * RC:1 * RC]
                    t1 = bb[:, 4 * RC:5 * RC]
                    axj, axj1 = T("axj"), T("axj1")
                    # axj = (j*hx-sx) * (1/sddx): per-partition scalar * tensor
                    nc.vector.tensor_scalar(axj[:], rsx[:], njc0[:, 0:1], None,
                                            AOP.mult)
                    nc.vector.tensor_scalar(axj1[:], rsx[:], njc1[:, 0:1],
                                            None, AOP.mult)
                    nc.vector.scalar_tensor_tensor(t0, axj[:], 1.0,
                                                   axj1[:], AOP.bypass,
                                                   AOP.min)
                    nc.vector.scalar_tensor_tensor(t0, t0, 1.0, amin,
                                                   AOP.bypass, AOP.max)
                    nc.vector.scalar_tensor_tensor(t1, axj[:], 1.0,
                                                   axj1[:], AOP.bypass,
                                                   AOP.max)
                    nc.vector.scalar_tensor_tensor(t1, t1, 1.0, amax,
                                                   AOP.bypass, AOP.min)

                    def counts(tt, al, be, off, outtag):
                        """C(t) = clamp(floor(t*al+be) + off, 0, 257)"""
                        gv, rv, fxs = T("cgv"), T("crv"), T("cfx")
                        nc.vector.scalar_tensor_tensor(gv[:], tt, 1.0,
                                                       al, AOP.bypass,
                                                       AOP.mult)
                        nc.vector.scalar_tensor_tensor(gv[:], gv[:], 1.0,
                                                       be, AOP.bypass, AOP.add)
                        nc.vector.tensor_scalar(rv[:], gv[:], float(MAGIC), float(MAGIC),
                                                AOP.add, AOP.subtract)
                        nc.vector.scalar_tensor_tensor(fxs[:], rv[:], 1.0,
                                                       gv[:], AOP.bypass,
                                                       AOP.is_gt)
                        out = T(outtag)
                        nc.vector.scalar_tensor_tensor(out[:], rv[:], 1.0,
                                                       fxs[:], AOP.bypass,
                                                       AOP.subtract)
                        nc.vector.scalar_tensor_tensor(out[:], out[:],
                                                       1.0, off,
                                                       AOP.bypass, AOP.add)
                        nc.vector.tensor_scalar(out[:], out[:], 0.0,
                                                257.0, AOP.max, AOP.min)
                        return out

                    def axis_events(al, be, off, Dv, m0, rs, hh, ss, tg):
                        C0 = counts(t0, al, be, off, "C0")
                        C1 = counts(t1, al, be, off, "C1")
                        cnt, m1, m2 = T("cnt"), T("m1"), T("m2")
                        nc.vector.scalar_tensor_tensor(cnt[:], C1[:], 1.0,
                                                       C0[:], AOP.bypass,
                                                       AOP.subtract)
                        nc.vector.scalar_tensor_tensor(m1[:], C0[:], 1.0,
                                                       Dv, AOP.bypass,
                                                       AOP.mult)
                        nc.vector.scalar_tensor_tensor(m1[:], m1[:], 1.0,
                                                       m0, AOP.bypass, AOP.add)
                        nc.vector.scalar_tensor_tensor(m2[:], m1[:], 1.0,
                                                       Dv, AOP.bypass, AOP.add)
                        e1, e2 = T(tg + "e1"), T(tg + "e2")
                        gg = T("gg", 1, mybir.dt.uint8)
                        for (mm, ee, thr) in ((m1, e1, 0.5),
                                              (m2, e2, 1.5)):
                            nc.vector.tensor_scalar(ee[:], mm[:], float(hh), float(-ss),
                                                    AOP.mult, AOP.add)
                            nc.vector.scalar_tensor_tensor(ee[:], ee[:],
                                                           1.0, rs[:],
                                                           AOP.bypass,
                                                           AOP.mult)
                            # invalid event -> amax sentinel (loses every min,
                            # clips to t1 => zero-width slot). Exact overwrite.
                            nc.vector.tensor_scalar(gg[:], cnt[:], thr, None,
                                                    AOP.is_le)
                            nc.vector.copy_predicated(ee[:], gg[:], amax)
                        return e1, e2

                    e1, e2 = axis_events(ay_, by_, offy, Dy, m0y, rsy, hy, sy,
                                         "y")
                    f1, f2 = axis_events(az_, bz_, offz, Dz, m0z, rsz, hz, sz,
                                         "z")

                    x1 = bb[:, 1 * RC:2 * RC]
                    x2 = bb[:, 2 * RC:3 * RC]
                    x3 = bb[:, 3 * RC:4 * RC]
                    hilo, lohi = T("hilo"), T("lohi")
                    nc.vector.scalar_tensor_tensor(x1, e1[:], 1.0, f1[:],
                                                   AOP.bypass, AOP.min)
                    nc.vector.scalar_tensor_tensor(hilo[:], e1[:], 1.0,
                                                   f1[:], AOP.bypass, AOP.max)
                    nc.vector.scalar_tensor_tensor(lohi[:], e2[:], 1.0,
                                                   f2[:], AOP.bypass, AOP.min)
                    nc.vector.scalar_tensor_tensor(x2, hilo[:], 1.0,
                                                   lohi[:], AOP.bypass,
                                                   AOP.min)
                    nc.vector.scalar_tensor_tensor(x3, hilo[:], 1.0,
                                                   lohi[:], AOP.bypass,
                                                   AOP.max)
                    for xx in (x1, x2, x3):
                        nc.vector.scalar_tensor_tensor(xx, xx, 1.0, t0,
                                                       AOP.bypass, AOP.max)
                        nc.vector.scalar_tensor_tensor(xx, xx, 1.0, t1,
                                                       AOP.bypass, AOP.min)

                    wgt = T("wgt", 4)
                    mid = T("mid", 4)
                    bL = bb[:, 0:4 * RC]
                    bR = bb[:, RC:5 * RC]
                    nc.vector.scalar_tensor_tensor(wgt[:], bR, 1.0, bL,
                                                   AOP.bypass, AOP.subtract)
                    nc.vector.tensor_scalar(wgt[:], wgt[:], 0.0, None,
                                            AOP.max)
                    nc.vector.scalar_tensor_tensor(mid[:], bL, 1.0, bR,
                                                   AOP.bypass, AOP.add)
                    nc.vector.tensor_scalar(mid[:], mid[:], 0.5, None,
                                            AOP.mult)

                    mid3 = mid[:].rearrange("p (four r) -> p four r", four=4)

                    def slot_floor_idx(sdd_p, inv_h, ss, outtag):
                        """floor((ss + mid*sdd)/h) clip [0,255] on [128,4RC]"""
                        pos, fxs = T("spos", 4), T("sfx", 4)
                        pos3 = pos[:].rearrange("p (four r) -> p four r",
                                                four=4)
                        sdd_b = sdd_p.rearrange(
                            "p (one r) -> p one r", one=1).broadcast_to(
                            (128, 4, RC))
                        nc.vector.scalar_tensor_tensor(pos3, mid3, 1.0,
                                                       sdd_b, AOP.bypass,
                                                       AOP.mult)
                        nc.vector.tensor_scalar(pos[:], pos[:], float(ss), float(inv_h),
                                                AOP.add, AOP.mult)
                        rv = T(outtag, 4)
                        nc.vector.tensor_scalar(rv[:], pos[:], float(MAGIC), float(MAGIC),
                                                AOP.add, AOP.subtract)
                        nc.vector.scalar_tensor_tensor(fxs[:], rv[:], 1.0,
                                                       pos[:], AOP.bypass,
                                                       AOP.is_gt)
                        nc.vector.scalar_tensor_tensor(rv[:], rv[:], 1.0,
                                                       fxs[:], AOP.bypass,
                                                       AOP.subtract)
                        nc.vector.tensor_scalar(rv[:], rv[:], 0.0,
                                                255.0, AOP.max, AOP.min)
                        return rv

                    iy = slot_floor_idx(sddy, inv_hy, sy, "iy")
                    iz = slot_floor_idx(sddz, inv_hz, sz, "iz")

                    izh, odd = T("izh", 4), T("odd", 4)
                    zh = T("zh", 4)
                    nc.vector.tensor_scalar(zh[:], iz[:], 0.5, None, AOP.mult)
                    nc.vector.tensor_scalar(izh[:], zh[:], float(MAGIC),
                                            float(MAGIC), AOP.add,
                                            AOP.subtract)
                    zfx = T("zfx", 4)
                    nc.vector.scalar_tensor_tensor(zfx[:], izh[:], 1.0, zh[:],
                                                   AOP.bypass, AOP.is_gt)
                    nc.vector.scalar_tensor_tensor(izh[:], izh[:], 1.0,
                                                   zfx[:], AOP.bypass,
                                                   AOP.subtract)
                    nc.vector.scalar_tensor_tensor(odd[:], izh[:], -2.0,
                                                   iz[:], AOP.mult, AOP.add)
                    pairf = T("pairf", 4)
                    nc.vector.scalar_tensor_tensor(pairf[:], iy[:], 128.0,
                                                   izh[:], AOP.mult, AOP.add)
                    idx16 = pool.tile([128, 4 * RC], mybir.dt.int16,
                                      tag="idx16")
                    nc.vector.tensor_copy(idx16[:], pairf[:])

                    gt = pool.tile([128, 4 * RC, 16, 2], mybir.dt.bfloat16,
                                   tag="gt")
                    nc.gpsimd.ap_gather(
                        out_ap=gt[:].rearrange("p a b c -> p (a b c)"),
                        in_ap=table[:], idxs_ap=idx16[:],
                        channels=128, num_elems=NE, d=2, num_idxs=16 * 4 * RC)

                    acc = T("acc", 4)
                    nc.vector.memset(acc[:], 0.0)
                    dvq, vq, wlom, mq = (T("dvq", 4), T("vq", 4),
                                         T("wlom", 4), T("mq", 4))
                    for q in range(16):
                        lo_q = gt[:, :, q, 0]
                        hi_q = gt[:, :, q, 1]
                        nc.vector.scalar_tensor_tensor(dvq[:], hi_q, 1.0,
                                                       lo_q, AOP.bypass,
                                                       AOP.subtract)
                        nc.vector.scalar_tensor_tensor(vq[:], dvq[:], 1.0,
                                                       odd[:], AOP.bypass,
                                                       AOP.mult)
                        nc.vector.scalar_tensor_tensor(vq[:], vq[:], 1.0,
                                                       lo_q, AOP.bypass,
                                                       AOP.add)
                        nc.vector.tensor_scalar(wlom[:], wgt[:],
                                                msk[:, q:q + 1], None,
                                                AOP.mult)
                        nc.vector.scalar_tensor_tensor(mq[:], vq[:], 1.0,
                                                       wlom[:], AOP.bypass,
                                                       AOP.mult)
                        nc.vector.scalar_tensor_tensor(acc[:], acc[:], 1.0,
                                                       mq[:], AOP.bypass,
                                                       AOP.add)
                    sch = T("sch")
                    nc.vector.tensor_reduce(
                        sch[:],
                        acc[:].rearrange("p (four r) -> p r four", four=4),
                        axis=mybir.AxisListType.X, op=AOP.add)
                    nc.sync.dma_start(s_hbm[g, :, ch * RC:(ch + 1) * RC],
                                      sch[:])

            # DRR[r] = rnorm[r] * sum_p (s0[p,r] + s1[p,r])
            ones = pool.tile([128, 1], mybir.dt.float32, tag="ones")
            nc.vector.memset(ones[:], 1.0)
            CH2 = 432
            for nb in range(NPAD // CH2):
                sl = slice(nb * CH2, (nb + 1) * CH2)
                s0 = pool.tile([128, CH2], mybir.dt.float32, tag="s0")
                s1 = pool.tile([128, CH2], mybir.dt.float32, tag="s1")
                nc.sync.dma_start(s0[:], s_hbm[0, :, sl])
                nc.sync.dma_start(s1[:], s_hbm[1, :, sl])
                nc.vector.scalar_tensor_tensor(s0[:], s0[:], 1.0, s1[:],
                                               AOP.bypass, AOP.add)
                ps = ppsum.tile([1, CH2], mybir.dt.float32)
                nc.tensor.matmul(ps[:], ones[:], s0[:], start=True, stop=True)
                rnc = pool.tile([1, CH2], mybir.dt.float32, tag="rnc")
                nc.sync.dma_start(rnc[:], rno_in[:, sl])
                orow = pool.tile([1, CH2], mybir.dt.float32, tag="orow")
                nc.vector.scalar_tensor_tensor(orow[:], ps[:], 1.0,
                                               rnc[:], AOP.bypass, AOP.mult)
                nc.sync.dma_start(drr_out[:, sl], orow[:])

    nc.finalize()
    return nc


def kernel(volume, spacing, sdr, rotations, translations):
    vol = np.asarray(volume, F32)
    par, rnorm, scal = _host_geometry(spacing, sdr, rotations, translations)
    if "nc" not in _cache:
        _cache["nc"] = _build(scal)
    nc = _cache["nc"]

    volf = vol[::-1].astype(bfloat16)                    # reference flip
    vol_dev = np.ascontiguousarray(volf.reshape(2, 128, NYV * NZV))

    in_maps = []
    for c in range(NCORES):
        lo = c * NPC
        pc = np.zeros((NP, NPAD), F32)
        pc[:, :NPC] = par[:, lo:lo + NPC]
        pc[:, NPC:] = pc[:, :1]                          # pad with ray 0
        pch = np.ascontiguousarray(
            pc.reshape(NP, NCH, RC).transpose(1, 0, 2)).reshape(NCH, 1,
                                                                NP * RC)
        pch = np.ascontiguousarray(np.broadcast_to(pch, (NCH, 128, NP * RC)))
        rn = np.zeros((1, NPAD), F32)
        rn[0, :NPC] = rnorm[lo:lo + NPC]
        mk = (np.arange(128)[:, None] % 16 ==
              np.arange(16)[None, :]).astype(F32)
        in_maps.append({"vol": vol_dev, "par": pch, "rno": rn, "msk": mk})

    _cache["in_maps"] = in_maps
    res = run_bass_kernel_spmd(nc, in_maps, list(range(NCORES)))
    drr = np.concatenate([res.results[c]["drr"][0, :NPC]
                          for c in range(NCORES)])
    return drr.reshape(1, H, W).astype(F32)
